# revision 5
# baseline (speedup 1.0000x reference)
"""Batched dynamic embedding table forward (gather + bag-sum pooling) on 8 trn2 cores.

Data-parallel over the batch; the [2097152, 64] f32 table is replicated to
every core's HBM. Core c handles samples [c*2048, (c+1)*2048).

The HW constraint that shapes the design: SWDGE descriptor generation costs
~8ns/descriptor of GpSimd Q7 time. Generic indirect DMAs (one instruction per
128 rows) issue SERIALLY on the Pool sequencer (~1.1us each, 800/core ->
~900us). The custom InstDMAGatherAnt issues to per-queue Q7 worker pairs and
overlaps ~3.2x across the 4 SWDGE queues -- but takes int16 indices, i.e. a
32768-row window per instruction.

So: two-phase gather (per core, 2048 samples x 50 bag, quarters of 512
samples):

Phase A: bucket the (sample,slot) pairs by (window w of 64, quarter q of 4)
  on the host; each (w,q) segment gets a fixed 512-slot allocation (pad with
  row 0; counts ~400+-20 << 512). 128 uniform 1024-idx dma_gathers (window-
  pure, queue round-robin) -> SBUF -> static HWDGE DMAs into DRAM staging,
  quarter-segregated: staging_q[w*512+pos] = row of pair (w,q,pos).
  Staging_q is exactly 32768 rows -> int16-addressable.

Phase B: per quarter, re-gather staging_q in slot-major sample order
  (i = k*512 + s_local): 25 x 1024-idx dma_gathers -> SBUF tile [128,200,64]
  where column c = 4k+b holds samples b*128+p on partition p. One strided DVE
  tensor_reduce per (q,b) pools the 50 slots; result rows are directly
  sample = q*512 + b*128 + p.

Host work is index-plan only (bucketing/int16 conversion); all value movement
and pooling is on-device.

Execution via bass2jax/PJRT shard_map with `values` replicated (staged once).
Fallback: the V1 indirect-DMA kernel, then run_bass_kernel_spmd.
"""

import os
from contextlib import contextmanager, nullcontext

import numpy as np

import concourse.bass as bass
import concourse.mybir as mybir
import concourse.tile as tile
from concourse import bacc
from concourse.bass import IndirectOffsetOnAxis

CAPACITY = 2097152
DIM = 64
BATCH = 16384
BAG = 50
N_CORES = 8
P = 128

SAMPLES_PER_CORE = BATCH // N_CORES  # 2048
N_WIN = 64          # 32768-row windows
WIN = 32768
N_QUARTER = 4       # sample quarters per core (512 samples each)
SPQ = SAMPLES_PER_CORE // N_QUARTER  # 512
SEG = 512           # fixed slots per (window, quarter) segment
PAIRS_Q = SPQ * BAG  # 25600 real pairs per quarter
A_INSTS = N_WIN * 2  # 128 x 1024-idx phase-A gathers
B_INSTS_Q = PAIRS_Q // 1024  # 25 per quarter

IMPL = os.environ.get("BASS_IMPL", "v2")


# Scoped patch for the V1 fallback: route indirect DMAs across SWDGE queues.
_Q_OVERRIDE = [None]
_ORIG_INSTDMA = mybir.InstDMACopy


def _patched_instdma(*a, **kw):
    if _Q_OVERRIDE[0] and kw.get("queue") == "qPoolDynamic":
        kw["queue"] = _Q_OVERRIDE[0]
    return _ORIG_INSTDMA(*a, **kw)


@contextmanager
def _queue_patch():
    mybir.InstDMACopy = _patched_instdma
    try:
        yield
    finally:
        mybir.InstDMACopy = _ORIG_INSTDMA


# ---------------------------------------------------------------------------
# V2 program


def build_nc(repeats=1, n_queues=4):
    if IMPL != "v2":
        return build_nc_v1(repeats=repeats, n_queues=n_queues)
    nc = bacc.Bacc(
        "TRN2",
        target_bir_lowering=False,
        debug=False,
        num_devices=N_CORES,
        num_swdge_queues=n_queues,
    )
    values = nc.dram_tensor(
        "values", [CAPACITY, DIM], mybir.dt.float32, kind="ExternalInput"
    ).ap()
    idxA = nc.dram_tensor(
        "idxA", [P, A_INSTS * 64], mybir.dt.int16, kind="ExternalInput"
    ).ap()
    idxB = nc.dram_tensor(
        "idxB", [P, N_QUARTER * B_INSTS_Q * 64], mybir.dt.int16, kind="ExternalInput"
    ).ap()
    out = nc.dram_tensor(
        "out", [SAMPLES_PER_CORE, DIM], mybir.dt.float32, kind="ExternalOutput"
    ).ap()

    with tile.TileContext(nc) as tc:
        with (
            tc.tile_pool(name="stg", space="DRAM", bufs=1) as spool,
            tc.tile_pool(name="idx", bufs=1) as ipool,
            tc.tile_pool(name="ga", bufs=6) as apool,
            tc.tile_pool(name="gb", bufs=2) as bpool,
            tc.tile_pool(name="out", bufs=4) as opool,
        ):
            stg = [
                spool.tile([N_WIN * SEG, DIM], mybir.dt.float32, name=f"stg{q}")
                for q in range(N_QUARTER)
            ]
            ia = ipool.tile([P, A_INSTS * 64], mybir.dt.int16)
            ib = ipool.tile([P, N_QUARTER * B_INSTS_Q * 64], mybir.dt.int16)
            rep_ctx = tc.For_i(0, repeats) if repeats > 1 else nullcontext()
            with rep_ctx:
                nc.sync.dma_start(out=ia[:], in_=idxA[:, :])
                nc.scalar.dma_start(out=ib[:], in_=idxB[:, :])
                # Phase A: window-pure gathers, quarters (0,1) first then (2,3)
                # so phase B of early quarters can overlap the tail of A.
                qi = 0
                for h in range(2):
                    for w in range(N_WIN):
                        j = w * 2 + h  # logical inst id; idxA col block
                        at = apool.tile([P, 8 * DIM], mybir.dt.float32)
                        at3 = at[:].rearrange("p (c d) -> p c d", d=DIM)
                        nc.gpsimd.dma_gather(
                            at3,
                            values[w * WIN : (w + 1) * WIN, :],
                            ia[:, j * 64 : (j + 1) * 64],
                            1024,
                            1024,
                            DIM,
                            queue_num=qi % n_queues,
                        )
                        qi += 1
                        for t in range(2):
                            q = 2 * h + t
                            eng = nc.sync if t == 0 else nc.scalar
                            eng.dma_start(
                                out=stg[q][:][w * SEG : (w + 1) * SEG, :].rearrange(
                                    "(c p) d -> p c d", p=P
                                ),
                                in_=at3[:, 4 * t : 4 * t + 4, :],
                            )
                # Phase B
                for q in range(N_QUARTER):
                    bt = bpool.tile([P, 4 * BAG * DIM], mybir.dt.float32)
                    bt3 = bt[:].rearrange("p (c d) -> p c d", d=DIM)
                    for m in range(B_INSTS_Q):
                        col = (q * B_INSTS_Q + m) * 64
                        nc.gpsimd.dma_gather(
                            bt3[:, 8 * m : 8 * m + 8, :],
                            stg[q][:][:, :],
                            ib[:, col : col + 64],
                            1024,
                            1024,
                            DIM,
                            queue_num=qi % n_queues,
                        )
                        qi += 1
                    b4 = bt[:].rearrange(
                        "p (k four d) -> p four d k", four=4, d=DIM
                    )
                    for b in range(4):
                        o = opool.tile([P, DIM], mybir.dt.float32)
                        nc.vector.tensor_reduce(
                            out=o[:],
                            in_=b4[:, b, :, :],
                            axis=mybir.AxisListType.X,
                            op=mybir.AluOpType.add,
                        )
                        nc.sync.dma_start(
                            out=out[q * SPQ + b * P : q * SPQ + (b + 1) * P, :],
                            in_=o[:],
                        )
    nc.compile()
    return nc


# ---------------------------------------------------------------------------
# V2 host-side index plan


def _plan_core(idx2d):
    """idx2d: [2048, 50] int -> (idxA [128, 8192] i16, idxB [128, 6400] i16)."""
    idx = np.asarray(idx2d, dtype=np.int64)
    n_pairs = SAMPLES_PER_CORE * BAG
    s = np.repeat(np.arange(SAMPLES_PER_CORE), BAG)
    w = (idx >> 15).ravel()
    local = (idx & 32767).ravel()
    q = s >> 9
    order = np.lexsort((q, w))
    wo, qo, lo = w[order], q[order], local[order]
    seg_id = wo * N_QUARTER + qo
    cnt = np.bincount(seg_id, minlength=N_WIN * N_QUARTER)
    if cnt.max() > SEG:
        raise ValueError(f"segment overflow: {cnt.max()} > {SEG}")
    seg_start = np.zeros(N_WIN * N_QUARTER, np.int64)
    seg_start[1:] = np.cumsum(cnt)[:-1]
    pos = np.arange(n_pairs) - seg_start[seg_id]
    A = np.zeros((N_WIN, N_QUARTER, SEG), np.int16)
    A[wo, qo, pos] = lo
    rank = np.empty(n_pairs, np.int32)
    rank[order] = (wo * SEG + pos).astype(np.int32)
    rank2d = rank.reshape(SAMPLES_PER_CORE, BAG)
    B = np.empty((N_QUARTER, PAIRS_Q), np.int16)
    for qq in range(N_QUARTER):
        blk = rank2d[qq * SPQ : (qq + 1) * SPQ, :]  # [512, 50]
        B[qq] = blk.T.ravel().astype(np.int16)      # i = k*512 + s_local
    # A instruction j = w*2 + h covers quarters (2h, 2h+1): [64,4,512] ->
    # [64, 2, 1024] -> [128, 1024] in (w-major, h-within) order; but program
    # emits h-major (all h=0 first). Reorder to emission order.
    Ai = A.reshape(N_WIN, 2, 1024)
    Ai = Ai[:, :, :]  # [w, h, 1024]
    # program's idxA col block for (w, h) is j = w*2 + h
    Ai = Ai.reshape(A_INSTS, 1024)
    Aw = Ai.reshape(A_INSTS, 64, 16).transpose(0, 2, 1)  # [inst, 16, 64]
    idxA16 = Aw.transpose(1, 0, 2).reshape(16, A_INSTS * 64)
    idxA = np.tile(idxA16, (8, 1))
    Bi = B.reshape(N_QUARTER * B_INSTS_Q, 1024)
    Bw = Bi.reshape(-1, 64, 16).transpose(0, 2, 1)
    idxB16 = Bw.transpose(1, 0, 2).reshape(16, N_QUARTER * B_INSTS_Q * 64)
    idxB = np.tile(idxB16, (8, 1))
    return np.ascontiguousarray(idxA), np.ascontiguousarray(idxB)


def _make_in_maps(values, indices):
    values = np.ascontiguousarray(np.asarray(values, dtype=np.float32))
    idx_all = np.asarray(indices).reshape(BATCH, BAG)
    in_maps = []
    for c in range(N_CORES):
        sl = idx_all[c * SAMPLES_PER_CORE : (c + 1) * SAMPLES_PER_CORE]
        if IMPL == "v2":
            idxA, idxB = _plan_core(sl)
            in_maps.append({"values": values, "idxA": idxA, "idxB": idxB})
        else:
            in_maps.append(
                {"values": values,
                 "idx": np.ascontiguousarray(sl.astype(np.int32))}
            )
    return in_maps


# ---------------------------------------------------------------------------
# V1 fallback program (indirect1d, known-good)


def build_nc_v1(repeats=1, n_queues=4):
    samples, bag, dim = SAMPLES_PER_CORE, BAG, DIM
    n_tiles = samples // P
    nc = bacc.Bacc(
        "TRN2",
        target_bir_lowering=False,
        debug=False,
        num_devices=N_CORES,
        num_swdge_queues=n_queues,
    )
    values = nc.dram_tensor(
        "values", [CAPACITY, dim], mybir.dt.float32, kind="ExternalInput"
    ).ap()
    idx = nc.dram_tensor(
        "idx", [samples, bag], mybir.dt.int32, kind="ExternalInput"
    ).ap()
    out = nc.dram_tensor(
        "out", [samples, dim], mybir.dt.float32, kind="ExternalOutput"
    ).ap()
    qi = 0
    with _queue_patch(), tile.TileContext(nc) as tc:
        with (
            tc.tile_pool(name="gather", bufs=3) as gpool,
            tc.tile_pool(name="idx", bufs=3) as ipool,
            tc.tile_pool(name="out", bufs=3) as opool,
        ):
            rep_ctx = tc.For_i(0, repeats) if repeats > 1 else nullcontext()
            with rep_ctx:
                for t in range(n_tiles):
                    rows = slice(t * P, (t + 1) * P)
                    idx_tile = ipool.tile([P, bag], mybir.dt.int32)
                    nc.sync.dma_start(out=idx_tile[:], in_=idx[rows, :])
                    g = gpool.tile([P, bag * dim], mybir.dt.float32)
                    for j in range(bag):
                        if n_queues > 1:
                            _Q_OVERRIDE[0] = f"qPoolDynamic{(qi % n_queues) or ''}"
                            qi += 1
                        nc.gpsimd.indirect_dma_start(
                            out=g[:, j * dim : (j + 1) * dim],
                            out_offset=None,
                            in_=values[:],
                            in_offset=IndirectOffsetOnAxis(
                                ap=idx_tile[:, j : j + 1], axis=0
                            ),
                        )
                        _Q_OVERRIDE[0] = None
                    o = opool.tile([P, dim], mybir.dt.float32)
                    nc.vector.tensor_reduce(
                        out=o[:],
                        in_=g[:].rearrange("p (j d) -> p d j", d=dim),
                        axis=mybir.AxisListType.X,
                        op=mybir.AluOpType.add,
                    )
                    nc.sync.dma_start(out=out[rows, :], in_=o[:])
    nc.compile()
    return nc


# ---------------------------------------------------------------------------
# Execution: bass2jax/PJRT with `values` replicated across the 8 cores.


class _Runner:
    def __init__(self, nc, n_cores, replicated=("values",)):
        import jax
        from jax.sharding import Mesh, PartitionSpec

        from concourse.bass2jax import (
            _bass_exec_p,
            install_neuronx_cc_hook,
            partition_id_tensor,
        )

        try:
            from jax.experimental.shard_map import shard_map
        except ImportError:
            shard_map = jax.shard_map

        install_neuronx_cc_hook()
        assert nc.dbg_addr is None
        self.jax = jax
        self.PartitionSpec = PartitionSpec
        self.nc = nc
        self.n_cores = n_cores
        self.replicated = set(replicated)
        partition_name = (
            nc.partition_id_tensor.name if nc.partition_id_tensor else None
        )

        in_names, out_names, out_avals = [], [], []
        for alloc in nc.m.functions[0].allocations:
            if not isinstance(alloc, mybir.MemoryLocationSet):
                continue
            name = alloc.memorylocations[0].name
            if alloc.kind == "ExternalInput":
                if name != partition_name:
                    in_names.append(name)
            elif alloc.kind == "ExternalOutput":
                out_names.append(name)
                out_avals.append(
                    jax.core.ShapedArray(
                        tuple(alloc.tensor_shape), mybir.dt.np(alloc.dtype)
                    )
                )
        self.in_names, self.out_names, self.out_avals = in_names, out_names, out_avals
        n_params = len(in_names)
        bind_names = in_names + out_names
        if partition_name is not None:
            bind_names = bind_names + [partition_name]

        def _body(*args):
            operands = list(args)
            if partition_name is not None:
                operands.append(partition_id_tensor())
            outs = _bass_exec_p.bind(
                *operands,
                out_avals=tuple(out_avals),
                in_names=tuple(bind_names),
                out_names=tuple(out_names),
                lowering_input_output_aliases=(),
                sim_require_finite=True,
                sim_require_nnan=True,
                nc=nc,
            )
            return tuple(outs)

        devices = jax.devices()[:n_cores]
        assert len(devices) >= n_cores
        self.mesh = Mesh(np.asarray(devices), ("core",))
        in_specs = tuple(
            PartitionSpec() if nm in self.replicated else PartitionSpec("core")
            for nm in in_names + out_names
        )
        out_specs = (PartitionSpec("core"),) * len(out_names)
        donate = tuple(range(n_params, n_params + len(out_names)))
        self.fn = jax.jit(
            shard_map(
                _body,
                mesh=self.mesh,
                in_specs=in_specs,
                out_specs=out_specs,
                check_rep=False,
            ),
            donate_argnums=donate,
            keep_unused=True,
        )

    def put_inputs(self, in_maps):
        from jax.sharding import NamedSharding, PartitionSpec

        args = []
        for nm in self.in_names:
            if nm in self.replicated:
                arr = np.asarray(in_maps[0][nm])
                sh = NamedSharding(self.mesh, PartitionSpec())
            else:
                arr = np.concatenate([np.asarray(m[nm]) for m in in_maps], axis=0)
                sh = NamedSharding(self.mesh, PartitionSpec("core"))
            args.append(self.jax.device_put(arr, sh))
        return args

    def _zeros(self):
        from jax.sharding import NamedSharding, PartitionSpec

        outs = []
        for av in self.out_avals:
            z = np.zeros((self.n_cores * av.shape[0], *av.shape[1:]), av.dtype)
            outs.append(
                self.jax.device_put(
                    z, NamedSharding(self.mesh, PartitionSpec("core"))
                )
            )
        return outs

    def run(self, dev_args):
        outs = self.fn(*dev_args, *self._zeros())
        return [np.asarray(o) for o in outs]


_CACHE = {}


def _get_nc():
    if "nc" not in _CACHE:
        _CACHE["nc"] = build_nc()
    return _CACHE["nc"]


def run_on_hw(values, indices):
    in_maps = _make_in_maps(values, indices)
    nc = _get_nc()
    if "runner" not in _CACHE:
        _CACHE["runner"] = _Runner(nc, N_CORES)
    r = _CACHE["runner"]
    dev = r.put_inputs(in_maps)
    outs = r.run(dev)
    out_idx = r.out_names.index("out")
    return outs[out_idx].reshape(BATCH, DIM)


def kernel(values, indices):
    global IMPL
    try:
        return run_on_hw(values, indices)
    except Exception:
        import traceback

        traceback.print_exc()
        if IMPL == "v2":
            # fall back to the known-good V1 program
            IMPL = "v1"
            _CACHE.clear()
            try:
                return run_on_hw(values, indices)
            except Exception:
                traceback.print_exc()
        from concourse.bass_utils import run_bass_kernel_spmd

        nc = _get_nc()
        res = run_bass_kernel_spmd(
            nc,
            _make_in_maps(values, indices),
            core_ids=list(range(N_CORES)),
        )
        return np.concatenate([r["out"] for r in res.results], axis=0)
